# revision 7
# baseline (speedup 1.0000x reference)
"""KVGather Trainium2 kernel.

Problem: out[n, i, k] = r_weight[n, i, k] * kv[n, r_idx[n, i, k]]
  r_idx:    (16, 64, 8)  int64, values in [0, 64)
  r_weight: (16, 64, 8)  float32
  kv:       (16, 64, 64, 128) float32
  out:      (16, 64, 8, 64, 128) float32

Strategy: data-parallel over batch n across 8 NeuronCores (2 batches/core).
Per core the output write dominates traffic; the rel-err budget (2e-2) is
spent to shrink it:
  - Device computes/stores the output in bf16 (~2^-9 rel err at every
    magnitude); the host casts back to f32.  Store traffic: 16.8 MB/core
    instead of 33.5 MB.  kv is kept in bf16.
  - The gather runs as one-hot matmuls with the f32 slot weight FOLDED
    into the selection matrix (S entries = w in bf16, not 1.0): PSUM
    already holds w*kv, so the drain is a pure f32->bf16 copy.  Total
    worst-case error (1+2^-9)^3 ~ 0.6%.
  - Contraction depth is only 64 (regions), so the 128x128 PE array is
    split into FOUR concurrent 64x64 tiles via tile_position: row half =
    batch, column half = slot group.
  - DVE/ACT alternate PSUM->bf16 copy drains; per-batch [128,1024] PSUM
    tiles (2 banks x 2 bufs x 2 tags = all 8 banks) keep the WAR
    rotation fine-grained.  Combined drain rate ~433 B/ns > fabric.
  - DMA queue split: kv loads issue from the ACT HWDGE ring, sel from
    the GpSimd SWDGE ring, stores from the SP HWDGE ring.  The 16 SDMA
    engines round-robin between rings at packet granularity, so the
    first store is NOT queued behind the kv loads (the old single-ring
    layout lost ~6 us of fabric time in the ramp).
  - kv loads as 3 chunks ([0:2], [2:4], [4:8] of the f dim) with 4-16KB
    rows: big-row transfers run near fabric rate, and chunk boundaries
    let the first matmuls start after only 512KB has landed.
  - Stores fire per f-quarter (16 stores of 1 MB, 8KB rows).

Layout per core (supertile st = 0..3 covers slots [st*128,(st+1)*128) of
BOTH batches):
  psum_b[64j+p, :] = sum_r S[b*64+r, st*2+j, p] * kv[b*64+r, :]
                   = w(slot)*kv[region]   (S carries the weight)
  stage[p, fq, b, f2] = psum_b[p, fq*2048+f2]                  (bf16)
  out_d[st, fq, p, b, f2] = out[batch b, slot st*128+p, fq*2048+f2]
"""

import sys

for _p in ("/opt/trn_rl_repo",):
    if _p not in sys.path:
        sys.path.insert(0, _p)

import numpy as np
import ml_dtypes

from concourse import bass, bacc, tile
from concourse import mybir
from concourse.bass_utils import run_bass_kernel_spmd

# Problem constants (hardcoded per contract)
N, P2, TOPK, W2, C_KV = 16, 64, 8, 64, 128
N_CORES = 8
B = N // N_CORES            # batches per core = 2
SLOTS = P2 * TOPK           # 512 output slots per batch
F = W2 * C_KV               # 8192 elements per region
ST = 4                      # supertiles; each = 128 slots x 2 batches
FC = 8                      # kv f-dim split granularity
F_PER_FC = F // FC          # 1024
TP = F // 1024              # 1024-wide f-pairs per supertile

_cached = {}


def _build_program():
    """Build the (input-independent) Bass program once."""
    if "nc" in _cached:
        return _cached["nc"]

    bf16 = mybir.dt.bfloat16
    f32 = mybir.dt.float32

    nc = bacc.Bacc()

    # kv plane: partition p = (batch p//64, region p%64); free (fc, elem).
    kv_d = nc.dram_tensor("kv", [128, FC, F_PER_FC], bf16, kind="ExternalInput")
    # Selection matrices: s_d[b*64+r, st*2+j, c] = w(slot) iff region r is
    # routed to batch b's slot st*128 + 64*j + c (weight folded in).
    s_d = nc.dram_tensor("sel", [128, ST * 2, 64], bf16, kind="ExternalInput")
    # out_d[st, fq, p, b, f2]: weighted gather of batch b's slot st*128+p,
    # f range fq*2048 + f2.
    out_d = nc.dram_tensor("out", [ST, 4, 128, B, F // 4], bf16, kind="ExternalOutput")

    with tile.TileContext(nc) as tc:
        with (
            tc.tile_pool(name="const", bufs=1) as const_pool,
            tc.tile_pool(name="kv", bufs=1) as kv_pool,
            tc.tile_pool(name="stage", bufs=4) as stage_pool,
            tc.tile_pool(name="psum", bufs=2, space=bass.MemorySpace.PSUM) as psum_pool,
        ):
            s_sb = const_pool.tile([128, ST * 2, 64], bf16, tag="sel")
            kv_sb = kv_pool.tile([128, FC, F_PER_FC], bf16, tag="kv")
            gate = const_pool.tile([1, 1], bf16, tag="gate")

            # Latency-critical small loads (sel for st0/st1, kv chunks
            # 0-1) head the SP HWDGE ring -- they complete by ~R+5us,
            # well before the first store needs that ring (FIFO per
            # ring), and the SP ring has ~0.2us inter-DMA gaps where the
            # ACT ring shows ~1.8us.  The bulk kv tail + late sel stream
            # via the GpSimd SWDGE ring, which round-robins with the SP
            # ring at packet granularity, so stores are never starved.
            nc.sync.dma_start(out=s_sb[:, 0:4], in_=s_d[:, 0:4])
            nc.sync.dma_start(out=kv_sb[:, 0:2], in_=kv_d[:, 0:2])
            # Gate the SWDGE bulk stream behind the critical loads: the
            # copy waits for kv[0:2], so the bulk packets don't steal
            # SDMA round-robin slots from sel/kv01 (measured: concurrent
            # rings stretch the critical 0.6 MB from 1.5us to 4us).
            nc.gpsimd.tensor_copy(gate[:], kv_sb[0:1, 1, 0:1])
            nc.gpsimd.dma_start(out=kv_sb[:, 2:4], in_=kv_d[:, 2:4])
            nc.gpsimd.dma_start(out=kv_sb[:, 4:6], in_=kv_d[:, 4:6])
            nc.gpsimd.dma_start(out=kv_sb[:, 6:8], in_=kv_d[:, 6:8])
            nc.gpsimd.dma_start(out=s_sb[:, 4:8], in_=s_d[:, 4:8])

            for st in range(ST):
                stage = stage_pool.tile([128, 4, B, F // 4], bf16, tag="stage")
                for tp in range(TP):
                    fq, tq = divmod(tp, 2)
                    ps = [
                        psum_pool.tile([128, 1024], f32, tag=f"ps{b}", name=f"ps{b}")
                        for b in range(B)
                    ]
                    for h in range(2):
                        off = h * 512
                        for b in range(B):
                            for j in range(2):
                                nc.tensor.matmul(
                                    ps[b][j * 64 : (j + 1) * 64, off : off + 512],
                                    s_sb[b * 64 : (b + 1) * 64, st * 2 + j, :],
                                    kv_sb[b * 64 : (b + 1) * 64, tp, off : off + 512],
                                    start=True,
                                    stop=True,
                                )
                    for b in range(B):
                        sl = stage[:, fq, b, tq * 1024 : (tq + 1) * 1024]
                        if b == 0:
                            nc.vector.tensor_copy(sl, ps[b][:])
                        else:
                            nc.scalar.copy(sl, ps[b][:])
                    if st == 0 and fq == 0:
                        # Prime the store stream: fire each half-fq (0.5 MB)
                        # of the very first quarter as soon as its two
                        # drains land, instead of waiting for the full MB.
                        nc.sync.dma_start(
                            out=out_d[0, 0, :, :, tq * 1024 : (tq + 1) * 1024],
                            in_=stage[:, 0, :, tq * 1024 : (tq + 1) * 1024],
                        )
                    elif tq == 1:
                        # Store the finished f-quarter: 128 rows x 8KB (1 MB).
                        nc.sync.dma_start(out=out_d[st, fq], in_=stage[:, fq])

    nc.compile()
    _cached["nc"] = nc
    return nc


def _prep_inputs(r_idx, r_weight, kv):
    """Shard + transform host inputs into per-core in_maps."""
    bf16 = ml_dtypes.bfloat16
    r_idx = np.asarray(r_idx).astype(np.int64)
    r_weight = np.asarray(r_weight).astype(np.float32)
    kv = np.asarray(kv).astype(np.float32).reshape(N, P2, F)

    kv_bf = kv.astype(bf16)

    in_maps = []
    for m in range(N_CORES):
        bsl = slice(m * B, (m + 1) * B)
        idx = r_idx[bsl].reshape(B, SLOTS)        # [2, 512] region ids
        wgt = r_weight[bsl].reshape(B, SLOTS)     # [2, 512] f32

        plane = kv_bf[bsl].reshape(128, FC, F_PER_FC)

        S = np.zeros((128, ST * 2, 64), dtype=bf16)
        cols = np.arange(64)
        for st in range(ST):
            for b in range(B):
                for j in range(2):
                    slots = st * 128 + 64 * j + cols
                    r = idx[b, slots]
                    S[b * 64 + r, st * 2 + j, cols] = wgt[b, slots].astype(bf16)

        in_maps.append({"kv": np.ascontiguousarray(plane), "sel": S})
    return in_maps


def _ensure_ntff_hook():
    """The agent image's antenv lacks axon_hooks, so the boot-time NTFF
    hook registration silently no-ops. Recreate the module and register
    the ctypes hook so trace=True yields exec_time_ns."""
    import types
    import antenv

    if "antenv.axon_hooks" in sys.modules:
        return
    mod = types.ModuleType("antenv.axon_hooks")
    _state = {"hook": None}
    mod.set_axon_ntff_profile_hook = lambda h: _state.__setitem__("hook", h)
    mod.get_axon_ntff_profile_hook = lambda: _state["hook"]
    sys.modules["antenv.axon_hooks"] = mod
    antenv.axon_hooks = mod
    try:
        if "/root/.axon_site" not in sys.path:
            sys.path.insert(0, "/root/.axon_site")
        from trn_agent_boot.trn_boot import _ntff_profile_via_ctypes

        hook = _ntff_profile_via_ctypes("/opt/axon/libaxon_pjrt.so")
        if hook is not None:
            mod.set_axon_ntff_profile_hook(hook)
    except Exception:
        pass


def kernel(r_idx, r_weight, kv, _trace=False, _trace_kwargs=None):
    if _trace:
        _ensure_ntff_hook()
    nc = _build_program()
    in_maps = _prep_inputs(r_idx, r_weight, kv)
    res = run_bass_kernel_spmd(
        nc,
        in_maps,
        core_ids=list(range(N_CORES)),
        trace=_trace,
        **(_trace_kwargs or {}),
    )
    out = np.empty((N, P2, TOPK, W2, C_KV), dtype=np.float32)
    for m in range(N_CORES):
        o = res.results[m]["out"]  # [ST, 4, 128, B, F//4] bf16
        o = np.asarray(o).astype(np.float32)
        # (st, fq, p, b, f2) -> (b, st*128+p, fq*2048+f2) = (batch, slot, f)
        o = np.transpose(o, (3, 0, 2, 1, 4)).reshape(B, SLOTS, F)
        out[m * B : (m + 1) * B] = o.reshape(B, P2, TOPK, W2, C_KV)
    if _trace:
        return out, res
    return out


if __name__ == "__main__":
    rng = np.random.default_rng(0)
    r_idx = rng.integers(0, P2, (N, P2, TOPK)).astype(np.int64)
    r_weight = rng.random((N, P2, TOPK), dtype=np.float32)
    kv = rng.standard_normal((N, P2, W2, C_KV), dtype=np.float32)
    out = kernel(r_idx, r_weight, kv)
    # local reference
    bidx = np.arange(N)[:, None, None]
    exp = r_weight[..., None, None] * kv[bidx, r_idx]
    err = np.abs(out - exp).max() / (np.abs(exp).max() + 1e-30)
    print("abs-rel err:", err)


# revision 9
# speedup vs baseline: 1.0311x; 1.0311x over previous
"""KVGather Trainium2 kernel.

Problem: out[n, i, k] = r_weight[n, i, k] * kv[n, r_idx[n, i, k]]
  r_idx:    (16, 64, 8)  int64, values in [0, 64)
  r_weight: (16, 64, 8)  float32
  kv:       (16, 64, 64, 128) float32
  out:      (16, 64, 8, 64, 128) float32

Strategy: data-parallel over batch n across 8 NeuronCores (2 batches/core).
Per core the output write dominates traffic; the rel-err budget (2e-2) is
spent to shrink it:
  - Device computes/stores the output in bf16 (~2^-9 rel err at every
    magnitude); the host casts back to f32.  Store traffic: 16.8 MB/core
    instead of 33.5 MB.  kv is kept in bf16.
  - The gather runs as one-hot matmuls with the f32 slot weight FOLDED
    into the selection matrix (S entries = w in bf16, not 1.0): PSUM
    already holds w*kv, so the drain is a pure f32->bf16 copy.  Total
    worst-case error (1+2^-9)^3 ~ 0.6%.
  - Contraction depth is only 64 (regions), so the 128x128 PE array is
    split into FOUR concurrent 64x64 tiles via tile_position: row half =
    batch, column half = slot group.
  - DVE/ACT alternate PSUM->bf16 copy drains; per-batch [128,1024] PSUM
    tiles (2 banks x 2 bufs x 2 tags = all 8 banks) keep the WAR
    rotation fine-grained.  Combined drain rate ~433 B/ns > fabric.
  - DMA queue split: kv loads issue from the ACT HWDGE ring, sel from
    the GpSimd SWDGE ring, stores from the SP HWDGE ring.  The 16 SDMA
    engines round-robin between rings at packet granularity, so the
    first store is NOT queued behind the kv loads (the old single-ring
    layout lost ~6 us of fabric time in the ramp).
  - kv loads as 3 chunks ([0:2], [2:4], [4:8] of the f dim) with 4-16KB
    rows: big-row transfers run near fabric rate, and chunk boundaries
    let the first matmuls start after only 512KB has landed.
  - Stores fire per f-quarter (16 stores of 1 MB, 8KB rows).

Layout per core (supertile st = 0..3 covers slots [st*128,(st+1)*128) of
BOTH batches):
  psum_b[64j+p, :] = sum_r S[b*64+r, st*2+j, p] * kv[b*64+r, :]
                   = w(slot)*kv[region]   (S carries the weight)
  stage[p, fq, b, f2] = psum_b[p, fq*2048+f2]                  (bf16)
  out_d[st, fq, p, b, f2] = out[batch b, slot st*128+p, fq*2048+f2]
"""

import sys

for _p in ("/opt/trn_rl_repo",):
    if _p not in sys.path:
        sys.path.insert(0, _p)

import numpy as np
import ml_dtypes

from concourse import bass, bacc, tile
from concourse import mybir
from concourse.bass_utils import run_bass_kernel_spmd

# Problem constants (hardcoded per contract)
N, P2, TOPK, W2, C_KV = 16, 64, 8, 64, 128
N_CORES = 8
B = N // N_CORES            # batches per core = 2
SLOTS = P2 * TOPK           # 512 output slots per batch
F = W2 * C_KV               # 8192 elements per region
ST = 4                      # supertiles; each = 128 slots x 2 batches
FC = 8                      # kv f-dim split granularity
F_PER_FC = F // FC          # 1024
TP = F // 1024              # 1024-wide f-pairs per supertile

_cached = {}


def _build_program():
    """Build the (input-independent) Bass program once."""
    if "nc" in _cached:
        return _cached["nc"]

    bf16 = mybir.dt.bfloat16
    f32 = mybir.dt.float32

    nc = bacc.Bacc()

    # kv plane: partition p = (batch p//64, region p%64); free (fc, elem).
    kv_d = nc.dram_tensor("kv", [128, FC, F_PER_FC], bf16, kind="ExternalInput")
    # Selection matrices: s_d[b*64+r, st*2+j, c] = w(slot) iff region r is
    # routed to batch b's slot st*128 + 64*j + c (weight folded in).
    s_d = nc.dram_tensor("sel", [128, ST * 2, 64], bf16, kind="ExternalInput")
    # out_d[st, fq, p, b, f2]: weighted gather of batch b's slot st*128+p,
    # f range fq*2048 + f2.
    out_d = nc.dram_tensor("out", [ST, 4, 128, B, F // 4], bf16, kind="ExternalOutput")

    with tile.TileContext(nc) as tc:
        with (
            tc.tile_pool(name="const", bufs=1) as const_pool,
            tc.tile_pool(name="kv", bufs=1) as kv_pool,
            tc.tile_pool(name="stage", bufs=4) as stage_pool,
            tc.tile_pool(name="psum", bufs=2, space=bass.MemorySpace.PSUM) as psum_pool,
        ):
            s_sb = const_pool.tile([128, ST * 2, 64], bf16, tag="sel")
            kv_sb = kv_pool.tile([128, FC, F_PER_FC], bf16, tag="kv")

            # Latency-critical small loads (sel for st0/st1, kv chunks
            # 0-1) head the SP HWDGE ring -- they complete by ~R+5us,
            # well before the first store needs that ring (FIFO per
            # ring), and the SP ring has ~0.2us inter-DMA gaps where the
            # ACT ring shows ~1.8us.  The bulk kv tail + late sel stream
            # via the GpSimd SWDGE ring, which round-robins with the SP
            # ring at packet granularity, so stores are never starved.
            # All loads head the SP HWDGE ring in consumption order; a
            # single ring avoids packet-granular round-robin contention
            # that stretches the latency-critical sel/kv01 loads
            # (measured: a concurrent bulk ring delays first-matmul by
            # ~3us).  Stores queue behind on the same ring but the last
            # load drains by ~R+9us, before store backlog matters.
            nc.sync.dma_start(out=s_sb[:, 0:4], in_=s_d[:, 0:4])
            nc.sync.dma_start(out=kv_sb[:, 0:2], in_=kv_d[:, 0:2])
            nc.sync.dma_start(out=kv_sb[:, 2:4], in_=kv_d[:, 2:4])
            nc.sync.dma_start(out=kv_sb[:, 4:8], in_=kv_d[:, 4:8])
            nc.sync.dma_start(out=s_sb[:, 4:8], in_=s_d[:, 4:8])

            for st in range(ST):
                stage = stage_pool.tile([128, 4, B, F // 4], bf16, tag="stage")
                for tp in range(TP):
                    fq, tq = divmod(tp, 2)
                    ps = [
                        psum_pool.tile([128, 1024], f32, tag=f"ps{b}", name=f"ps{b}")
                        for b in range(B)
                    ]
                    for h in range(2):
                        off = h * 512
                        for b in range(B):
                            for j in range(2):
                                nc.tensor.matmul(
                                    ps[b][j * 64 : (j + 1) * 64, off : off + 512],
                                    s_sb[b * 64 : (b + 1) * 64, st * 2 + j, :],
                                    kv_sb[b * 64 : (b + 1) * 64, tp, off : off + 512],
                                    start=True,
                                    stop=True,
                                )
                    for b in range(B):
                        sl = stage[:, fq, b, tq * 1024 : (tq + 1) * 1024]
                        if b == 0:
                            nc.vector.tensor_copy(sl, ps[b][:])
                        else:
                            nc.scalar.copy(sl, ps[b][:])
                    if st == 0 and fq == 0:
                        # Prime the store stream: fire each half-fq (0.5 MB)
                        # of the very first quarter as soon as its two
                        # drains land, instead of waiting for the full MB.
                        nc.sync.dma_start(
                            out=out_d[0, 0, :, :, tq * 1024 : (tq + 1) * 1024],
                            in_=stage[:, 0, :, tq * 1024 : (tq + 1) * 1024],
                        )
                    elif tq == 1:
                        # Store the finished f-quarter: 128 rows x 8KB (1 MB).
                        nc.sync.dma_start(out=out_d[st, fq], in_=stage[:, fq])

    nc.compile()
    _cached["nc"] = nc
    return nc


def _prep_inputs(r_idx, r_weight, kv):
    """Shard + transform host inputs into per-core in_maps."""
    bf16 = ml_dtypes.bfloat16
    r_idx = np.asarray(r_idx).astype(np.int64)
    r_weight = np.asarray(r_weight).astype(np.float32)
    kv = np.asarray(kv).astype(np.float32).reshape(N, P2, F)

    kv_bf = kv.astype(bf16)

    in_maps = []
    for m in range(N_CORES):
        bsl = slice(m * B, (m + 1) * B)
        idx = r_idx[bsl].reshape(B, SLOTS)        # [2, 512] region ids
        wgt = r_weight[bsl].reshape(B, SLOTS)     # [2, 512] f32

        plane = kv_bf[bsl].reshape(128, FC, F_PER_FC)

        S = np.zeros((128, ST * 2, 64), dtype=bf16)
        cols = np.arange(64)
        for st in range(ST):
            for b in range(B):
                for j in range(2):
                    slots = st * 128 + 64 * j + cols
                    r = idx[b, slots]
                    S[b * 64 + r, st * 2 + j, cols] = wgt[b, slots].astype(bf16)

        in_maps.append({"kv": np.ascontiguousarray(plane), "sel": S})
    return in_maps


def _ensure_ntff_hook():
    """The agent image's antenv lacks axon_hooks, so the boot-time NTFF
    hook registration silently no-ops. Recreate the module and register
    the ctypes hook so trace=True yields exec_time_ns."""
    import types
    import antenv

    if "antenv.axon_hooks" in sys.modules:
        return
    mod = types.ModuleType("antenv.axon_hooks")
    _state = {"hook": None}
    mod.set_axon_ntff_profile_hook = lambda h: _state.__setitem__("hook", h)
    mod.get_axon_ntff_profile_hook = lambda: _state["hook"]
    sys.modules["antenv.axon_hooks"] = mod
    antenv.axon_hooks = mod
    try:
        if "/root/.axon_site" not in sys.path:
            sys.path.insert(0, "/root/.axon_site")
        from trn_agent_boot.trn_boot import _ntff_profile_via_ctypes

        hook = _ntff_profile_via_ctypes("/opt/axon/libaxon_pjrt.so")
        if hook is not None:
            mod.set_axon_ntff_profile_hook(hook)
    except Exception:
        pass


def kernel(r_idx, r_weight, kv, _trace=False, _trace_kwargs=None):
    if _trace:
        _ensure_ntff_hook()
    nc = _build_program()
    in_maps = _prep_inputs(r_idx, r_weight, kv)
    res = run_bass_kernel_spmd(
        nc,
        in_maps,
        core_ids=list(range(N_CORES)),
        trace=_trace,
        **(_trace_kwargs or {}),
    )
    out = np.empty((N, P2, TOPK, W2, C_KV), dtype=np.float32)
    for m in range(N_CORES):
        o = res.results[m]["out"]  # [ST, 4, 128, B, F//4] bf16
        o = np.asarray(o).astype(np.float32)
        # (st, fq, p, b, f2) -> (b, st*128+p, fq*2048+f2) = (batch, slot, f)
        o = np.transpose(o, (3, 0, 2, 1, 4)).reshape(B, SLOTS, F)
        out[m * B : (m + 1) * B] = o.reshape(B, P2, TOPK, W2, C_KV)
    if _trace:
        return out, res
    return out


if __name__ == "__main__":
    rng = np.random.default_rng(0)
    r_idx = rng.integers(0, P2, (N, P2, TOPK)).astype(np.int64)
    r_weight = rng.random((N, P2, TOPK), dtype=np.float32)
    kv = rng.standard_normal((N, P2, W2, C_KV), dtype=np.float32)
    out = kernel(r_idx, r_weight, kv)
    # local reference
    bidx = np.arange(N)[:, None, None]
    exp = r_weight[..., None, None] * kv[bidx, r_idx]
    err = np.abs(out - exp).max() / (np.abs(exp).max() + 1e-30)
    print("abs-rel err:", err)


# revision 12
# speedup vs baseline: 1.1200x; 1.0862x over previous
"""KVGather Trainium2 kernel.

Problem: out[n, i, k] = r_weight[n, i, k] * kv[n, r_idx[n, i, k]]
  r_idx:    (16, 64, 8)  int64, values in [0, 64)
  r_weight: (16, 64, 8)  float32
  kv:       (16, 64, 64, 128) float32
  out:      (16, 64, 8, 64, 128) float32

Strategy: data-parallel over batch n across 8 NeuronCores (2 batches/core).
Per core the output write dominates traffic; the rel-err budget (2e-2) is
spent to shrink it:
  - Device computes/stores the output in bf16 (~2^-9 rel err at every
    magnitude); the host casts back to f32.  Store traffic: 16.8 MB/core
    instead of 33.5 MB.  kv is kept in bf16.
  - The gather runs as one-hot matmuls with the f32 slot weight FOLDED
    into the selection matrix (S entries = w in bf16, not 1.0): PSUM
    already holds w*kv, so the drain is a pure f32->bf16 copy.  Total
    worst-case error (1+2^-9)^3 ~ 0.6%.
  - Contraction depth is only 64 (regions), so the 128x128 PE array is
    split into FOUR concurrent 64x64 tiles via tile_position: row half =
    batch, column half = slot group.
  - DVE/ACT alternate PSUM->bf16 copy drains; per-batch [128,1024] PSUM
    tiles (2 banks x 2 bufs x 2 tags = all 8 banks) keep the WAR
    rotation fine-grained.  Combined drain rate ~433 B/ns >= fabric.
  - sel and kv are PACKED into one DRAM tensor so the whole input loads
    as THREE ring DMAs ([sel|kv01], [kv23], [kv4567]).  Each extra DMA
    on a HWDGE ring costs ~0.9us of completion-receipt stall before the
    next starts; 3 DMAs instead of 8-10 pulls first-matmul in by ~2us
    and frees the ring for stores by ~R+9us.
  - Stores fire per f-quarter (16 stores of 1 MB, 8KB rows) on the same
    SP ring, except the first quarter which is split in half so the
    store stream starts as soon as the first two drains land.

Layout per core (supertile st = 0..3 covers slots [st*128,(st+1)*128) of
BOTH batches):
  psum_b[64j+p, :] = sum_r S[b*64+r, st*2+j, p] * kv[b*64+r, :]
                   = w(slot)*kv[region]   (S carries the weight)
  stage[p, fq, b, f2] = psum_b[p, fq*2048+f2]                  (bf16)
  out_d[st, fq, p, b, f2] = out[batch b, slot st*128+p, fq*2048+f2]
"""

import sys

for _p in ("/opt/trn_rl_repo",):
    if _p not in sys.path:
        sys.path.insert(0, _p)

import numpy as np
import ml_dtypes

from concourse import bass, bacc, tile
from concourse import mybir
from concourse.bass_utils import run_bass_kernel_spmd

# Problem constants (hardcoded per contract)
N, P2, TOPK, W2, C_KV = 16, 64, 8, 64, 128
N_CORES = 8
B = N // N_CORES            # batches per core = 2
SLOTS = P2 * TOPK           # 512 output slots per batch
F = W2 * C_KV               # 8192 elements per region
ST = 4                      # supertiles; each = 128 slots x 2 batches
TP = F // 1024              # 1024-wide f-pairs per supertile
SEL = ST * 2 * 64           # 512 packed sel columns
PACK = SEL + F              # 8704 packed input columns per partition

_cached = {}


def _build_program():
    """Build the (input-independent) Bass program once."""
    if "nc" in _cached:
        return _cached["nc"]

    bf16 = mybir.dt.bfloat16
    f32 = mybir.dt.float32

    nc = bacc.Bacc()

    # Packed input plane: partition p = (batch p//64, region p%64);
    # cols [0:512]  = selection matrices s[st2j*64 + c] with the f32
    #                 slot weight folded in (bf16),
    # cols [512:]   = kv for that (batch, region) row.
    in_d = nc.dram_tensor("inp", [128, PACK], bf16, kind="ExternalInput")
    # out_d[st, fq, p, b, f2]: weighted gather of batch b's slot st*128+p,
    # f range fq*2048 + f2.
    out_d = nc.dram_tensor("out", [ST, 4, 128, B, F // 4], bf16, kind="ExternalOutput")

    with tile.TileContext(nc) as tc:
        with (
            tc.tile_pool(name="inp", bufs=1) as in_pool,
            tc.tile_pool(name="stage", bufs=4) as stage_pool,
            tc.tile_pool(name="psum", bufs=2, space=bass.MemorySpace.PSUM) as psum_pool,
        ):
            in_sb = in_pool.tile([128, PACK], bf16, tag="inp")

            def sel_ap(b, st, j):
                c0 = (st * 2 + j) * 64
                return in_sb[b * 64 : (b + 1) * 64, c0 : c0 + 64]

            def kv_ap(b, tp, off):
                c0 = SEL + tp * 1024 + off
                return in_sb[b * 64 : (b + 1) * 64, c0 : c0 + 512]

            # Three loads head the SP HWDGE ring in consumption order
            # (stores queue behind them; the ring is free by ~R+9us).
            # The first carries only sel+kv0 so the first matmul starts
            # as early as possible.
            nc.sync.dma_start(out=in_sb[:, 0 : SEL + 1024], in_=in_d[:, 0 : SEL + 1024])
            nc.sync.dma_start(
                out=in_sb[:, SEL + 1024 : SEL + 4096],
                in_=in_d[:, SEL + 1024 : SEL + 4096],
            )
            nc.sync.dma_start(out=in_sb[:, SEL + 4096 :], in_=in_d[:, SEL + 4096 :])

            for st in range(ST):
                stage = stage_pool.tile([128, 4, B, F // 4], bf16, tag="stage")
                for tp in range(TP):
                    fq, tq = divmod(tp, 2)
                    ps = [
                        psum_pool.tile([128, 1024], f32, tag=f"ps{b}", name=f"ps{b}")
                        for b in range(B)
                    ]
                    for h in range(2):
                        off = h * 512
                        for b in range(B):
                            for j in range(2):
                                nc.tensor.matmul(
                                    ps[b][j * 64 : (j + 1) * 64, off : off + 512],
                                    sel_ap(b, st, j),
                                    kv_ap(b, tp, off),
                                    start=True,
                                    stop=True,
                                )
                    for b in range(B):
                        sl = stage[:, fq, b, tq * 1024 : (tq + 1) * 1024]
                        if b == 0:
                            nc.vector.tensor_copy(sl, ps[b][:])
                        else:
                            nc.scalar.copy(sl, ps[b][:])
                    if st == 0 and fq == 0:
                        # Prime the store stream: fire each half-fq (0.5 MB)
                        # of the very first quarter as soon as its two
                        # drains land, instead of waiting for the full MB.
                        nc.sync.dma_start(
                            out=out_d[0, 0, :, :, tq * 1024 : (tq + 1) * 1024],
                            in_=stage[:, 0, :, tq * 1024 : (tq + 1) * 1024],
                        )
                    elif tq == 1:
                        # Store the finished f-quarter: 128 rows x 8KB (1 MB).
                        nc.sync.dma_start(out=out_d[st, fq], in_=stage[:, fq])

    nc.compile()
    _cached["nc"] = nc
    return nc


def _prep_inputs(r_idx, r_weight, kv):
    """Shard + transform host inputs into per-core in_maps."""
    bf16 = ml_dtypes.bfloat16
    r_idx = np.asarray(r_idx).astype(np.int64)
    r_weight = np.asarray(r_weight).astype(np.float32)
    kv = np.asarray(kv).astype(np.float32).reshape(N, P2, F)

    kv_bf = kv.astype(bf16)

    in_maps = []
    for m in range(N_CORES):
        bsl = slice(m * B, (m + 1) * B)
        idx = r_idx[bsl].reshape(B, SLOTS)        # [2, 512] region ids
        wgt = r_weight[bsl].reshape(B, SLOTS)     # [2, 512] f32

        plane = kv_bf[bsl].reshape(128, F)

        S = np.zeros((128, ST * 2, 64), dtype=bf16)
        cols = np.arange(64)
        for st in range(ST):
            for b in range(B):
                for j in range(2):
                    slots = st * 128 + 64 * j + cols
                    r = idx[b, slots]
                    S[b * 64 + r, st * 2 + j, cols] = wgt[b, slots].astype(bf16)

        packed = np.concatenate([S.reshape(128, SEL), plane], axis=1)
        in_maps.append({"inp": np.ascontiguousarray(packed)})
    return in_maps


def _ensure_ntff_hook():
    """The agent image's antenv lacks axon_hooks, so the boot-time NTFF
    hook registration silently no-ops. Recreate the module and register
    the ctypes hook so trace=True yields exec_time_ns."""
    import types
    import antenv

    if "antenv.axon_hooks" in sys.modules:
        return
    mod = types.ModuleType("antenv.axon_hooks")
    _state = {"hook": None}
    mod.set_axon_ntff_profile_hook = lambda h: _state.__setitem__("hook", h)
    mod.get_axon_ntff_profile_hook = lambda: _state["hook"]
    sys.modules["antenv.axon_hooks"] = mod
    antenv.axon_hooks = mod
    try:
        if "/root/.axon_site" not in sys.path:
            sys.path.insert(0, "/root/.axon_site")
        from trn_agent_boot.trn_boot import _ntff_profile_via_ctypes

        hook = _ntff_profile_via_ctypes("/opt/axon/libaxon_pjrt.so")
        if hook is not None:
            mod.set_axon_ntff_profile_hook(hook)
    except Exception:
        pass


def kernel(r_idx, r_weight, kv, _trace=False, _trace_kwargs=None):
    if _trace:
        _ensure_ntff_hook()
    nc = _build_program()
    in_maps = _prep_inputs(r_idx, r_weight, kv)
    res = run_bass_kernel_spmd(
        nc,
        in_maps,
        core_ids=list(range(N_CORES)),
        trace=_trace,
        **(_trace_kwargs or {}),
    )
    out = np.empty((N, P2, TOPK, W2, C_KV), dtype=np.float32)
    for m in range(N_CORES):
        o = res.results[m]["out"]  # [ST, 4, 128, B, F//4] bf16
        o = np.asarray(o).astype(np.float32)
        # (st, fq, p, b, f2) -> (b, st*128+p, fq*2048+f2) = (batch, slot, f)
        o = np.transpose(o, (3, 0, 2, 1, 4)).reshape(B, SLOTS, F)
        out[m * B : (m + 1) * B] = o.reshape(B, P2, TOPK, W2, C_KV)
    if _trace:
        return out, res
    return out


if __name__ == "__main__":
    rng = np.random.default_rng(0)
    r_idx = rng.integers(0, P2, (N, P2, TOPK)).astype(np.int64)
    r_weight = rng.random((N, P2, TOPK), dtype=np.float32)
    kv = rng.standard_normal((N, P2, W2, C_KV), dtype=np.float32)
    out = kernel(r_idx, r_weight, kv)
    # local reference
    bidx = np.arange(N)[:, None, None]
    exp = r_weight[..., None, None] * kv[bidx, r_idx]
    err = np.abs(out - exp).max() / (np.abs(exp).max() + 1e-30)
    print("abs-rel err:", err)
